# revision 1
# baseline (speedup 1.0000x reference)
"""Multi-head attention (B=8, S=2048, E=1024, H=8, D=128) on 8 Trainium2 cores.

Strategy: data-parallel over batch (one batch element per core, no collectives).
Per core, heads are processed in pairs:
  - QT/KT per head ([d, s] layout) and V ([t, d] layout, split per head) are
    computed from a transposed-activation slab streamed from DRAM.
  - Attention runs in the transposed-score layout S^T = [t, s] so softmax(P) @ V
    needs no transposes: PV matmuls contract t on partitions directly.
  - Softmax skips the max-subtraction (scores are O(1) here, exp is safe) and
    gets row sums via a ones-matmul into PSUM (replicated across partitions);
    normalization happens on the [d, s] attention output (128x smaller than
    normalizing the probability matrix).
  - Output projection contracts the concatenated head dim with Wo^T. The q bias
    is fused into the PSUM eviction (per-partition tensor_scalar_add); v and o
    biases are K=1 rank-1 matmuls; bk is dropped: it adds a per-query constant
    to every score row, which softmax cancels exactly.
Per-head buffers rotate (bufs=1 per head-tag) so the next pair's projections
overlap the ACT-bound attention of the current pair's second head.
All matmuls run as float32r (fp32 storage, reduced-precision multiply) which
measures ~bf16 speed at ~16x better accuracy; 256-wide moving-dim tiles are
used (measured ~2x per-column PE throughput vs 512-wide on this hardware).
"""

import numpy as np
from contextlib import ExitStack

import concourse.bass as bass
import concourse.tile as tile
from concourse import bacc, mybir
from concourse.bass_utils import run_bass_kernel_spmd

B = 8
S = 2048
E = 1024
H = 8
D = 128
P = 128
EC = E // P          # 8 contraction chunks over embed
TC = S // P          # 16 t-chunks
NSB = S // 512       # 4 s-blocks of 512
SCALE = 1.0 / float(np.sqrt(D))

F32 = mybir.dt.float32
F32R = mybir.dt.float32r
EXP = mybir.ActivationFunctionType.Exp


def build_nc(reps=1, st256=False, qfuse=True, do_out=True, att_mode=2, exp_copy=False, do_rs=True, out2x=False, rs_mode='pe', esp_bufs=4):
    nc = bacc.Bacc(None)

    xT = nc.dram_tensor("xT", (E, S), F32R, kind="ExternalInput")
    WqT = nc.dram_tensor("WqT", (E, E), F32R, kind="ExternalInput")
    WkT = nc.dram_tensor("WkT", (E, E), F32R, kind="ExternalInput")
    WvT = nc.dram_tensor("WvT", (E, E), F32R, kind="ExternalInput")
    WoT = nc.dram_tensor("WoT", (E, E), F32R, kind="ExternalInput")
    bq2 = nc.dram_tensor("bq2", (P, H), F32, kind="ExternalInput")   # bq as [d, h]
    bvr = nc.dram_tensor("bvr", (1, E), F32R, kind="ExternalInput")
    bor = nc.dram_tensor("bor", (1, E), F32R, kind="ExternalInput")
    onesd = nc.dram_tensor("onesd", (P, 512), F32R, kind="ExternalInput")
    out = nc.dram_tensor("out", (S, E), F32, kind="ExternalOutput")

    x_r = xT.rearrange("(ec p) s -> p ec s", p=P)
    wq_r = WqT.rearrange("(ec p) d -> p ec d", p=P)
    wk_r = WkT.rearrange("(ec p) d -> p ec d", p=P)
    wv_r = WvT.rearrange("(ec p) d -> p ec d", p=P)
    wo_r = WoT.rearrange("(hc p) e -> p hc e", p=P)

    with tile.TileContext(nc) as tc:
        with ExitStack() as octx:
            const = octx.enter_context(tc.tile_pool(name="const", bufs=1))
            atp = octx.enter_context(tc.tile_pool(name="atp", bufs=1))

            ones = const.tile([P, 512], F32R)   # [:,0:128] lhsT block; [0:1,:] rank-1 rows
            nc.sync.dma_start(out=ones, in_=onesd[:, :])
            bq_s = const.tile([P, H], F32)
            nc.sync.dma_start(out=bq_s, in_=bq2[:, :])
            bv_s = const.tile([1, E], F32R)
            nc.sync.dma_start(out=bv_s, in_=bvr[:, :])
            bo_s = const.tile([1, E], F32R)
            nc.sync.dma_start(out=bo_s, in_=bor[:, :])
            ones_f = const.tile([P, P], F32)
            nc.vector.memset(ones_f, 1.0)

            AT = atp.tile([P, H, S], F32R)  # normalized A^T per head

            for _rep in range(reps):
                with ExitStack() as ctx:
                    wpool = ctx.enter_context(tc.tile_pool(name="wpool", bufs=1))
                    slabp = ctx.enter_context(tc.tile_pool(name="slabp", bufs=2))
                    qkp = ctx.enter_context(tc.tile_pool(name="qkp", bufs=1))
                    vp = ctx.enter_context(tc.tile_pool(name="vp", bufs=1))
                    esp = ctx.enter_context(tc.tile_pool(name="esp", bufs=esp_bufs))
                    rcpp = ctx.enter_context(tc.tile_pool(name="rcpp", bufs=2))
                    rap = ctx.enter_context(tc.tile_pool(name="rap", bufs=2))
                    stp = ctx.enter_context(tc.tile_pool(name="stp", bufs=2, space="PSUM"))
                    accp = ctx.enter_context(tc.tile_pool(
                        name="accp", bufs=(3 if rs_mode == "pe512" else 4), space="PSUM"))
                    rsp = (ctx.enter_context(tc.tile_pool(name="rsp", bufs=1, space="PSUM"))
                           if rs_mode == "pe512" else None)

                    for pair in range(H // 2):
                        h0 = 2 * pair
                        wq0 = wpool.tile([P, EC, D], F32R, tag="wq0")
                        wq1 = wpool.tile([P, EC, D], F32R, tag="wq1")
                        wk0 = wpool.tile([P, EC, D], F32R, tag="wk0")
                        wk1 = wpool.tile([P, EC, D], F32R, tag="wk1")
                        wv01 = wpool.tile([P, EC, 2 * D], F32R, tag="wv01")
                        nc.sync.dma_start(out=wq0, in_=wq_r[:, :, h0 * D:(h0 + 1) * D])
                        nc.sync.dma_start(out=wq1, in_=wq_r[:, :, (h0 + 1) * D:(h0 + 2) * D])
                        nc.sync.dma_start(out=wk0, in_=wk_r[:, :, h0 * D:(h0 + 1) * D])
                        nc.sync.dma_start(out=wk1, in_=wk_r[:, :, (h0 + 1) * D:(h0 + 2) * D])
                        nc.sync.dma_start(out=wv01, in_=wv_r[:, :, h0 * D:(h0 + 2) * D])

                        qt0 = qkp.tile([P, S], F32R, tag="qt0")
                        qt1 = qkp.tile([P, S], F32R, tag="qt1")
                        kt0 = qkp.tile([P, S], F32R, tag="kt0")
                        kt1 = qkp.tile([P, S], F32R, tag="kt1")
                        vv0 = vp.tile([P, TC, D], F32R, tag="vv0")
                        vv1 = vp.tile([P, TC, D], F32R, tag="vv1")

                        # ---- QKV projections for the pair ----
                        for sb in range(NSB):
                            slab = slabp.tile([P, EC, 512], F32R, tag="slab")
                            nc.sync.dma_start(out=slab,
                                              in_=x_r[:, :, sb * 512:(sb + 1) * 512])

                            for w, dest, hh in ((wq0, qt0, h0), (wq1, qt1, h0 + 1),
                                                (wk0, kt0, None), (wk1, kt1, None)):
                                ps = accp.tile([P, 512], F32, tag="acc",
                                               name=f"ps_qk{pair}_{sb}_{hh}")
                                for half in range(2):
                                    n0 = half * 256
                                    for ec in range(EC):
                                        nc.tensor.matmul(
                                            ps[:, n0:n0 + 256],
                                            w[:, ec, :],
                                            slab[:, ec, n0:n0 + 256],
                                            start=(half == 0 and ec == 0),
                                            stop=(half == 1 and ec == EC - 1),
                                        )
                                if hh is not None and qfuse:
                                    nc.vector.tensor_scalar_add(
                                        dest[:, sb * 512:(sb + 1) * 512], ps,
                                        bq_s[:, hh:hh + 1])
                                elif hh is not None:
                                    nc.vector.tensor_scalar_add(
                                        dest[:, sb * 512:(sb + 1) * 512], ps,
                                        bq_s[:, hh:hh + 1])
                                else:
                                    nc.vector.tensor_copy(
                                        dest[:, sb * 512:(sb + 1) * 512], ps)

                            # V for both heads: [t_chunk, 2D]; two t-chunks per bank
                            for tcp in range(2):
                                ps = accp.tile([P, 512], F32, tag="acc",
                                               name=f"ps_v{pair}_{sb}_{tcp}")
                                for j in range(2):
                                    n0 = j * 256
                                    t0 = (tcp * 2 + j) * P
                                    for ec in range(EC):
                                        nc.tensor.matmul(
                                            ps[:, n0:n0 + 256],
                                            slab[:, ec, t0:t0 + P],
                                            wv01[:, ec, :],
                                            start=(j == 0 and ec == 0), stop=False,
                                        )
                                    nc.tensor.matmul(  # v bias: rank-1 ones ⊗ bv
                                        ps[:, n0:n0 + 256],
                                        ones[0:1, 0:P],
                                        bv_s[0:1, h0 * D:(h0 + 2) * D],
                                        start=False, stop=(j == 1),
                                    )
                                for j in range(2):
                                    tc_i = sb * 4 + tcp * 2 + j
                                    nc.vector.tensor_copy(vv0[:, tc_i, :],
                                                          ps[:, j * 256:j * 256 + D])
                                    nc.vector.tensor_copy(vv1[:, tc_i, :],
                                                          ps[:, j * 256 + D:(j + 1) * 256])

                        # ---- attention for each head of the pair ----
                        for hi in range(2 if att_mode >= 1 else 0):
                            h = h0 + hi
                            qt = (qt0, qt1)[hi]
                            kt = (kt0, kt1)[hi]
                            vv = (vv0, vv1)[hi]
                            for sb in range(NSB):
                                s0 = sb * 512
                                ps_pv = accp.tile([P, 512], F32, tag="acc",
                                                  name=f"pv{pair}_{hi}_{sb}")
                                if rs_mode == "pe":
                                    ps_rs = accp.tile([P, 512], F32, tag="acc",
                                                      name=f"rs{pair}_{hi}_{sb}")
                                elif rs_mode == "pe512":
                                    ps_rs = rsp.tile([P, 512], F32, tag="rs",
                                                     name=f"rs{pair}_{hi}_{sb}")
                                else:
                                    racc_d = rap.tile([P, 512], F32, tag="racc_d",
                                                      name=f"rad{pair}_{hi}_{sb}")
                                    racc_g = rap.tile([P, 512], F32, tag="racc_g",
                                                      name=f"rag{pair}_{hi}_{sb}")
                                for tcp in range(TC // 2):
                                    st = stp.tile([P, 1024], F32, tag="st",
                                                  name=f"st{pair}_{hi}_{sb}_{tcp}")
                                    for j in range(2):
                                        tc_i = tcp * 2 + j
                                        if st256:
                                            for m in range(2):
                                                nc.tensor.matmul(
                                                    st[:, j * 512 + m * 256:j * 512 + (m + 1) * 256],
                                                    kt[:, tc_i * P:(tc_i + 1) * P],
                                                    qt[:, s0 + m * 256:s0 + (m + 1) * 256],
                                                    start=(m == 0), stop=(m == 1),
                                                )
                                        else:
                                            nc.tensor.matmul(
                                                st[:, j * 512:(j + 1) * 512],
                                                kt[:, tc_i * P:(tc_i + 1) * P],
                                                qt[:, s0:s0 + 512],
                                                start=True, stop=True,
                                            )
                                    es = esp.tile([P, 1024], F32R, tag="es",
                                                  name=f"es{pair}_{hi}_{sb}_{tcp}")
                                    nc.scalar.activation(
                                        es, st,
                                        mybir.ActivationFunctionType.Copy if exp_copy else EXP,
                                        scale=SCALE)
                                    if att_mode < 2:
                                        nc.vector.tensor_copy(AT[:, h, s0:s0 + 512], es[:, 0:512])
                                        continue
                                    for j in range(2):
                                        tc_i = tcp * 2 + j
                                        for half in range(2):
                                            n0 = j * 512 + half * 256
                                            o0 = half * 256
                                            nc.tensor.matmul(
                                                ps_pv[:, o0:o0 + 256],
                                                vv[:, tc_i, :],
                                                es[:, n0:n0 + 256],
                                                start=(tc_i == 0 and half == 0),
                                                stop=(tc_i == TC - 1 and half == 1),
                                            )
                                    if do_rs and rs_mode == "pe512":
                                        for j in range(2):
                                            nc.tensor.matmul(
                                                ps_rs[:, 0:512],
                                                ones[:, 0:P],
                                                es[:, j * 512:(j + 1) * 512],
                                                start=(tcp == 0 and j == 0),
                                                stop=(tcp == TC // 2 - 1 and j == 1),
                                            )
                                    elif do_rs and rs_mode == "pe":
                                        for j in range(2):
                                            tc_i = tcp * 2 + j
                                            for half in range(2):
                                                n0 = j * 512 + half * 256
                                                o0 = half * 256
                                                nc.tensor.matmul(
                                                    ps_rs[:, o0:o0 + 256],
                                                    ones[:, 0:P],
                                                    es[:, n0:n0 + 256],
                                                    start=(tc_i == tcp * 2 and half == 0 and tcp == 0),
                                                    stop=(tc_i == TC - 1 and half == 1),
                                                )
                                    elif do_rs:
                                        eng = nc.vector if tcp < 4 else nc.gpsimd
                                        racc = racc_d if tcp < 4 else racc_g
                                        if tcp % 4 == 0:
                                            eng.tensor_add(racc, es[:, 0:512].bitcast(F32),
                                                           es[:, 512:1024].bitcast(F32))
                                        else:
                                            eng.tensor_add(racc, racc, es[:, 0:512].bitcast(F32))
                                            eng.tensor_add(racc, racc, es[:, 512:1024].bitcast(F32))
                                if att_mode >= 2:
                                    rcp = rcpp.tile([P, 512], F32, tag="rcp",
                                                    name=f"rcp{pair}_{hi}_{sb}")
                                    if not do_rs:
                                        nc.vector.reciprocal(rcp, ps_pv)
                                    elif rs_mode in ("pe", "pe512"):
                                        nc.vector.reciprocal(rcp, ps_rs)
                                    else:
                                        nc.vector.tensor_add(racc_d, racc_d, racc_g)
                                        rsb = accp.tile([P, 512], F32, tag="acc",
                                                        name=f"rsb{pair}_{hi}_{sb}")
                                        nc.tensor.matmul(rsb[:, :],
                                                         ones_f[:, :],
                                                         racc_d[:, :],
                                                         start=True, stop=True)
                                        nc.vector.reciprocal(rcp, rsb)
                                    nc.vector.tensor_mul(AT[:, h, s0:s0 + 512], ps_pv, rcp)

                # ---- output projection ----
                if not do_out:
                    continue
                with ExitStack() as ctx:
                    wop = ctx.enter_context(tc.tile_pool(name="wop", bufs=1))
                    outp = ctx.enter_context(tc.tile_pool(name="outp", bufs=4))
                    pso = ctx.enter_context(tc.tile_pool(name="pso", bufs=4, space="PSUM"))

                    wo = wop.tile([P, H, E], F32R)
                    nc.sync.dma_start(out=wo, in_=wo_r[:, :, :])

                    for _o2 in range(2 if out2x else 1):
                      for sc in range(S // P):
                        ps = pso.tile([P, E], F32, tag="po", name=f"po{sc}")
                        for hc in range(H):  # lhsT loaded once per (hc, sc)
                            for ebq in range(4):
                                nc.tensor.matmul(
                                    ps[:, ebq * 256:(ebq + 1) * 256],
                                    AT[:, hc, sc * P:(sc + 1) * P],
                                    wo[:, hc, ebq * 256:(ebq + 1) * 256],
                                    start=(hc == 0 and ebq % 2 == 0), stop=False,
                                )
                        for ebq in range(4):  # bias: rank-1 ones ⊗ bo
                            nc.tensor.matmul(
                                ps[:, ebq * 256:(ebq + 1) * 256],
                                ones[0:1, 0:P],
                                bo_s[0:1, ebq * 256:(ebq + 1) * 256],
                                start=False, stop=(ebq % 2 == 1),
                            )
                        ot = outp.tile([P, E], F32, tag="ot", name=f"ot{sc}")
                        nc.vector.tensor_copy(ot, ps)
                        nc.sync.dma_start(out=out[sc * P:(sc + 1) * P, :], in_=ot)

    nc.compile()
    return nc


_NC_CACHE = []


def _get_nc():
    if not _NC_CACHE:
        _NC_CACHE.append(build_nc())
    return _NC_CACHE[0]


def kernel(hidden_state, Wq, bq, Wk, bk, Wv, bv, Wo, bo):
    hidden_state = np.ascontiguousarray(hidden_state, dtype=np.float32)
    WqT = np.ascontiguousarray(np.asarray(Wq, np.float32).reshape(E, E).T)
    WkT = np.ascontiguousarray(np.asarray(Wk, np.float32).reshape(E, E).T)
    WvT = np.ascontiguousarray(np.asarray(Wv, np.float32).reshape(E, E).T)
    WoT = np.ascontiguousarray(np.asarray(Wo, np.float32).T)
    bq2a = np.ascontiguousarray(np.asarray(bq, np.float32).reshape(H, D).T)
    bvr = np.asarray(bv, np.float32).reshape(1, E)
    bor = np.asarray(bo, np.float32).reshape(1, E)
    ones = np.ones((P, 512), np.float32)
    # bk is mathematically a no-op through softmax (per-query constant shift).

    nc = _get_nc()
    in_maps = []
    for c in range(B):
        xT = np.ascontiguousarray(hidden_state[c].T)
        in_maps.append({
            "xT": xT, "WqT": WqT, "WkT": WkT, "WvT": WvT, "WoT": WoT,
            "bq2": bq2a, "bvr": bvr, "bor": bor, "onesd": ones,
        })
    res = run_bass_kernel_spmd(nc, in_maps, core_ids=list(range(B)))
    return np.stack([res.results[c]["out"] for c in range(B)])



# revision 2
# speedup vs baseline: 24.4924x; 24.4924x over previous
"""Multi-head attention (B=8, S=2048, E=1024, H=8, D=128) on 8 Trainium2 cores.

Strategy: data-parallel over batch (one batch element per core, no collectives).
Per core, heads are processed in pairs:
  - QT/KT per head ([d, s] layout) and V ([t, d] layout, split per head) are
    computed from a transposed-activation slab streamed from DRAM.
  - Attention runs in the transposed-score layout S^T = [t, s] so softmax(P) @ V
    needs no transposes: PV matmuls contract t on partitions directly.
  - Softmax skips the max-subtraction (scores are O(1) here, exp is safe) and
    gets row sums via a ones-matmul into PSUM (replicated across partitions);
    normalization happens on the [d, s] attention output (128x smaller than
    normalizing the probability matrix).
  - Output projection contracts the concatenated head dim with Wo^T. The q bias
    is fused into the PSUM eviction (per-partition tensor_scalar_add); v and o
    biases are K=1 rank-1 matmuls; bk is dropped: it adds a per-query constant
    to every score row, which softmax cancels exactly.
Per-head buffers rotate (bufs=1 per head-tag) so the next pair's projections
overlap the ACT-bound attention of the current pair's second head.
All matmuls run as float32r (fp32 storage, reduced-precision multiply) which
measures ~bf16 speed at ~16x better accuracy; 256-wide moving-dim tiles are
used (measured ~2x per-column PE throughput vs 512-wide on this hardware).
"""

import numpy as np
from contextlib import ExitStack

import concourse.bass as bass
import concourse.tile as tile
from concourse import bacc, mybir
from concourse.bass_utils import run_bass_kernel_spmd

B = 8
S = 2048
E = 1024
H = 8
D = 128
P = 128
EC = E // P          # 8 contraction chunks over embed
TC = S // P          # 16 t-chunks
NSB = S // 512       # 4 s-blocks of 512
SCALE = 1.0 / float(np.sqrt(D))

F32 = mybir.dt.float32
F32R = mybir.dt.float32r
EXP = mybir.ActivationFunctionType.Exp


def build_nc(reps=1, st256=False, qfuse=True, do_out=True, att_mode=2, exp_copy=False, do_rs=True, out2x=False, rs_mode='pe', esp_bufs=4):
    nc = bacc.Bacc(None)

    xT = nc.dram_tensor("xT", (E, S), F32R, kind="ExternalInput")
    WqT = nc.dram_tensor("WqT", (E, E), F32R, kind="ExternalInput")
    WkT = nc.dram_tensor("WkT", (E, E), F32R, kind="ExternalInput")
    WvT = nc.dram_tensor("WvT", (E, E), F32R, kind="ExternalInput")
    WoT = nc.dram_tensor("WoT", (E, E), F32R, kind="ExternalInput")
    bq2 = nc.dram_tensor("bq2", (P, H), F32, kind="ExternalInput")   # bq as [d, h]
    bvr = nc.dram_tensor("bvr", (1, E), F32R, kind="ExternalInput")
    bor = nc.dram_tensor("bor", (1, E), F32R, kind="ExternalInput")
    onesd = nc.dram_tensor("onesd", (P, 512), F32R, kind="ExternalInput")
    out = nc.dram_tensor("out", (S, E), F32, kind="ExternalOutput")

    x_r = xT.rearrange("(ec p) s -> p ec s", p=P)
    wq_r = WqT.rearrange("(ec p) d -> p ec d", p=P)
    wk_r = WkT.rearrange("(ec p) d -> p ec d", p=P)
    wv_r = WvT.rearrange("(ec p) d -> p ec d", p=P)
    wo_r = WoT.rearrange("(hc p) e -> p hc e", p=P)

    with tile.TileContext(nc) as tc:
        with ExitStack() as octx:
            const = octx.enter_context(tc.tile_pool(name="const", bufs=1))
            atp = octx.enter_context(tc.tile_pool(name="atp", bufs=1))

            ones = const.tile([P, 512], F32R)   # [:,0:128] lhsT block; [0:1,:] rank-1 rows
            nc.sync.dma_start(out=ones, in_=onesd[:, :])
            bq_s = const.tile([P, H], F32)
            nc.sync.dma_start(out=bq_s, in_=bq2[:, :])
            bv_s = const.tile([1, E], F32R)
            nc.sync.dma_start(out=bv_s, in_=bvr[:, :])
            bo_s = const.tile([1, E], F32R)
            nc.sync.dma_start(out=bo_s, in_=bor[:, :])
            ones_f = const.tile([P, P], F32)
            nc.vector.memset(ones_f, 1.0)

            AT = atp.tile([P, H, S], F32R)  # normalized A^T per head

            for _rep in range(reps):
                with ExitStack() as ctx:
                    wpool = ctx.enter_context(tc.tile_pool(name="wpool", bufs=1))
                    slabp = ctx.enter_context(tc.tile_pool(name="slabp", bufs=2))
                    qkp = ctx.enter_context(tc.tile_pool(name="qkp", bufs=1))
                    vp = ctx.enter_context(tc.tile_pool(name="vp", bufs=1))
                    esp = ctx.enter_context(tc.tile_pool(name="esp", bufs=esp_bufs))
                    rcpp = ctx.enter_context(tc.tile_pool(name="rcpp", bufs=2))
                    rap = ctx.enter_context(tc.tile_pool(name="rap", bufs=2))
                    stp = ctx.enter_context(tc.tile_pool(name="stp", bufs=2, space="PSUM"))
                    accp = ctx.enter_context(tc.tile_pool(
                        name="accp", bufs=(3 if rs_mode == "pe512" else 4), space="PSUM"))
                    rsp = (ctx.enter_context(tc.tile_pool(name="rsp", bufs=1, space="PSUM"))
                           if rs_mode == "pe512" else None)

                    for pair in range(H // 2):
                        h0 = 2 * pair
                        wq0 = wpool.tile([P, EC, D], F32R, tag="wq0")
                        wq1 = wpool.tile([P, EC, D], F32R, tag="wq1")
                        wk0 = wpool.tile([P, EC, D], F32R, tag="wk0")
                        wk1 = wpool.tile([P, EC, D], F32R, tag="wk1")
                        wv01 = wpool.tile([P, EC, 2 * D], F32R, tag="wv01")
                        nc.sync.dma_start(out=wq0, in_=wq_r[:, :, h0 * D:(h0 + 1) * D])
                        nc.sync.dma_start(out=wq1, in_=wq_r[:, :, (h0 + 1) * D:(h0 + 2) * D])
                        nc.sync.dma_start(out=wk0, in_=wk_r[:, :, h0 * D:(h0 + 1) * D])
                        nc.sync.dma_start(out=wk1, in_=wk_r[:, :, (h0 + 1) * D:(h0 + 2) * D])
                        nc.sync.dma_start(out=wv01, in_=wv_r[:, :, h0 * D:(h0 + 2) * D])

                        qt0 = qkp.tile([P, S], F32R, tag="qt0")
                        qt1 = qkp.tile([P, S], F32R, tag="qt1")
                        kt0 = qkp.tile([P, S], F32R, tag="kt0")
                        kt1 = qkp.tile([P, S], F32R, tag="kt1")
                        vv0 = vp.tile([P, TC, D], F32R, tag="vv0")
                        vv1 = vp.tile([P, TC, D], F32R, tag="vv1")

                        # ---- QKV projections for the pair ----
                        for sb in range(NSB):
                            slab = slabp.tile([P, EC, 512], F32R, tag="slab")
                            nc.sync.dma_start(out=slab,
                                              in_=x_r[:, :, sb * 512:(sb + 1) * 512])

                            for w, dest, hh in ((wq0, qt0, h0), (wq1, qt1, h0 + 1),
                                                (wk0, kt0, None), (wk1, kt1, None)):
                                ps = accp.tile([P, 512], F32, tag="acc",
                                               name=f"ps_qk{pair}_{sb}_{hh}")
                                for half in range(2):
                                    n0 = half * 256
                                    for ec in range(EC):
                                        nc.tensor.matmul(
                                            ps[:, n0:n0 + 256],
                                            w[:, ec, :],
                                            slab[:, ec, n0:n0 + 256],
                                            start=(half == 0 and ec == 0),
                                            stop=(half == 1 and ec == EC - 1),
                                        )
                                if hh is not None and qfuse:
                                    nc.vector.tensor_scalar_add(
                                        dest[:, sb * 512:(sb + 1) * 512], ps,
                                        bq_s[:, hh:hh + 1])
                                elif hh is not None:
                                    nc.vector.tensor_scalar_add(
                                        dest[:, sb * 512:(sb + 1) * 512], ps,
                                        bq_s[:, hh:hh + 1])
                                else:
                                    nc.vector.tensor_copy(
                                        dest[:, sb * 512:(sb + 1) * 512], ps)

                            # V for both heads: [t_chunk, 2D]; two t-chunks per bank
                            for tcp in range(2):
                                ps = accp.tile([P, 512], F32, tag="acc",
                                               name=f"ps_v{pair}_{sb}_{tcp}")
                                for j in range(2):
                                    n0 = j * 256
                                    t0 = (tcp * 2 + j) * P
                                    for ec in range(EC):
                                        nc.tensor.matmul(
                                            ps[:, n0:n0 + 256],
                                            slab[:, ec, t0:t0 + P],
                                            wv01[:, ec, :],
                                            start=(j == 0 and ec == 0), stop=False,
                                        )
                                    nc.tensor.matmul(  # v bias: rank-1 ones ⊗ bv
                                        ps[:, n0:n0 + 256],
                                        ones[0:1, 0:P],
                                        bv_s[0:1, h0 * D:(h0 + 2) * D],
                                        start=False, stop=(j == 1),
                                    )
                                for j in range(2):
                                    tc_i = sb * 4 + tcp * 2 + j
                                    nc.vector.tensor_copy(vv0[:, tc_i, :],
                                                          ps[:, j * 256:j * 256 + D])
                                    nc.vector.tensor_copy(vv1[:, tc_i, :],
                                                          ps[:, j * 256 + D:(j + 1) * 256])

                        # ---- attention for each head of the pair ----
                        for hi in range(2 if att_mode >= 1 else 0):
                            h = h0 + hi
                            qt = (qt0, qt1)[hi]
                            kt = (kt0, kt1)[hi]
                            vv = (vv0, vv1)[hi]
                            for sb in range(NSB):
                                s0 = sb * 512
                                ps_pv = accp.tile([P, 512], F32, tag="acc",
                                                  name=f"pv{pair}_{hi}_{sb}")
                                if rs_mode == "pe":
                                    ps_rs = accp.tile([P, 512], F32, tag="acc",
                                                      name=f"rs{pair}_{hi}_{sb}")
                                elif rs_mode == "pe512":
                                    ps_rs = rsp.tile([P, 512], F32, tag="rs",
                                                     name=f"rs{pair}_{hi}_{sb}")
                                else:
                                    racc_d = rap.tile([P, 512], F32, tag="racc_d",
                                                      name=f"rad{pair}_{hi}_{sb}")
                                    racc_g = rap.tile([P, 512], F32, tag="racc_g",
                                                      name=f"rag{pair}_{hi}_{sb}")
                                for tcp in range(TC // 2):
                                    st = stp.tile([P, 1024], F32, tag="st",
                                                  name=f"st{pair}_{hi}_{sb}_{tcp}")
                                    for j in range(2):
                                        tc_i = tcp * 2 + j
                                        if st256:
                                            for m in range(2):
                                                nc.tensor.matmul(
                                                    st[:, j * 512 + m * 256:j * 512 + (m + 1) * 256],
                                                    kt[:, tc_i * P:(tc_i + 1) * P],
                                                    qt[:, s0 + m * 256:s0 + (m + 1) * 256],
                                                    start=(m == 0), stop=(m == 1),
                                                )
                                        else:
                                            nc.tensor.matmul(
                                                st[:, j * 512:(j + 1) * 512],
                                                kt[:, tc_i * P:(tc_i + 1) * P],
                                                qt[:, s0:s0 + 512],
                                                start=True, stop=True,
                                            )
                                    es = esp.tile([P, 1024], F32R, tag="es",
                                                  name=f"es{pair}_{hi}_{sb}_{tcp}")
                                    nc.scalar.activation(
                                        es, st,
                                        mybir.ActivationFunctionType.Copy if exp_copy else EXP,
                                        scale=SCALE)
                                    if att_mode < 2:
                                        nc.vector.tensor_copy(AT[:, h, s0:s0 + 512], es[:, 0:512])
                                        continue
                                    for j in range(2):
                                        tc_i = tcp * 2 + j
                                        for half in range(2):
                                            n0 = j * 512 + half * 256
                                            o0 = half * 256
                                            nc.tensor.matmul(
                                                ps_pv[:, o0:o0 + 256],
                                                vv[:, tc_i, :],
                                                es[:, n0:n0 + 256],
                                                start=(tc_i == 0 and half == 0),
                                                stop=(tc_i == TC - 1 and half == 1),
                                            )
                                    if do_rs and rs_mode == "pe512":
                                        for j in range(2):
                                            nc.tensor.matmul(
                                                ps_rs[:, 0:512],
                                                ones[:, 0:P],
                                                es[:, j * 512:(j + 1) * 512],
                                                start=(tcp == 0 and j == 0),
                                                stop=(tcp == TC // 2 - 1 and j == 1),
                                            )
                                    elif do_rs and rs_mode == "pe":
                                        for j in range(2):
                                            tc_i = tcp * 2 + j
                                            for half in range(2):
                                                n0 = j * 512 + half * 256
                                                o0 = half * 256
                                                nc.tensor.matmul(
                                                    ps_rs[:, o0:o0 + 256],
                                                    ones[:, 0:P],
                                                    es[:, n0:n0 + 256],
                                                    start=(tc_i == tcp * 2 and half == 0 and tcp == 0),
                                                    stop=(tc_i == TC - 1 and half == 1),
                                                )
                                    elif do_rs:
                                        eng = nc.vector if tcp < 4 else nc.gpsimd
                                        racc = racc_d if tcp < 4 else racc_g
                                        if tcp % 4 == 0:
                                            eng.tensor_add(racc, es[:, 0:512].bitcast(F32),
                                                           es[:, 512:1024].bitcast(F32))
                                        else:
                                            eng.tensor_add(racc, racc, es[:, 0:512].bitcast(F32))
                                            eng.tensor_add(racc, racc, es[:, 512:1024].bitcast(F32))
                                if att_mode >= 2:
                                    rcp = rcpp.tile([P, 512], F32, tag="rcp",
                                                    name=f"rcp{pair}_{hi}_{sb}")
                                    if not do_rs:
                                        nc.vector.reciprocal(rcp, ps_pv)
                                    elif rs_mode in ("pe", "pe512"):
                                        nc.vector.reciprocal(rcp, ps_rs)
                                    else:
                                        nc.vector.tensor_add(racc_d, racc_d, racc_g)
                                        rsb = accp.tile([P, 512], F32, tag="acc",
                                                        name=f"rsb{pair}_{hi}_{sb}")
                                        nc.tensor.matmul(rsb[:, :],
                                                         ones_f[:, :],
                                                         racc_d[:, :],
                                                         start=True, stop=True)
                                        nc.vector.reciprocal(rcp, rsb)
                                    nc.vector.tensor_mul(AT[:, h, s0:s0 + 512], ps_pv, rcp)

                # ---- output projection ----
                if not do_out:
                    continue
                with ExitStack() as ctx:
                    wop = ctx.enter_context(tc.tile_pool(name="wop", bufs=1))
                    outp = ctx.enter_context(tc.tile_pool(name="outp", bufs=4))
                    pso = ctx.enter_context(tc.tile_pool(name="pso", bufs=4, space="PSUM"))

                    wo = wop.tile([P, H, E], F32R)
                    nc.sync.dma_start(out=wo, in_=wo_r[:, :, :])

                    for _o2 in range(2 if out2x else 1):
                      for sc in range(S // P):
                        ps = pso.tile([P, E], F32, tag="po", name=f"po{sc}")
                        for hc in range(H):  # lhsT loaded once per (hc, sc)
                            for ebq in range(4):
                                nc.tensor.matmul(
                                    ps[:, ebq * 256:(ebq + 1) * 256],
                                    AT[:, hc, sc * P:(sc + 1) * P],
                                    wo[:, hc, ebq * 256:(ebq + 1) * 256],
                                    start=(hc == 0 and ebq % 2 == 0), stop=False,
                                )
                        for ebq in range(4):  # bias: rank-1 ones ⊗ bo
                            nc.tensor.matmul(
                                ps[:, ebq * 256:(ebq + 1) * 256],
                                ones[0:1, 0:P],
                                bo_s[0:1, ebq * 256:(ebq + 1) * 256],
                                start=False, stop=(ebq % 2 == 1),
                            )
                        ot = outp.tile([P, E], F32, tag="ot", name=f"ot{sc}")
                        nc.vector.tensor_copy(ot, ps)
                        nc.sync.dma_start(out=out[sc * P:(sc + 1) * P, :], in_=ot)

    nc.compile()
    return nc


_NC_CACHE = []


def _get_nc():
    if not _NC_CACHE:
        _NC_CACHE.append(build_nc())
    return _NC_CACHE[0]


def prep_inmaps(hidden_state, Wq, bq, Wk, bk, Wv, bv, Wo, bo):
    hidden_state = np.ascontiguousarray(hidden_state, dtype=np.float32)
    WqT = np.ascontiguousarray(np.asarray(Wq, np.float32).reshape(E, E).T)
    WkT = np.ascontiguousarray(np.asarray(Wk, np.float32).reshape(E, E).T)
    WvT = np.ascontiguousarray(np.asarray(Wv, np.float32).reshape(E, E).T)
    WoT = np.ascontiguousarray(np.asarray(Wo, np.float32).T)
    bq2a = np.ascontiguousarray(np.asarray(bq, np.float32).reshape(H, D).T)
    bvr = np.asarray(bv, np.float32).reshape(1, E)
    bor = np.asarray(bo, np.float32).reshape(1, E)
    ones = np.ones((P, 512), np.float32)
    # bk is mathematically a no-op through softmax (per-query constant shift).
    in_maps = []
    for c in range(B):
        xT = np.ascontiguousarray(hidden_state[c].T)
        in_maps.append({
            "xT": xT, "WqT": WqT, "WkT": WkT, "WvT": WvT, "WoT": WoT,
            "bq2": bq2a, "bvr": bvr, "bor": bor, "onesd": ones,
        })
    return in_maps


def kernel(hidden_state, Wq, bq, Wk, bk, Wv, bv, Wo, bo):
    in_maps = prep_inmaps(hidden_state, Wq, bq, Wk, bk, Wv, bv, Wo, bo)
    nc = _get_nc()
    res = run_bass_kernel_spmd(nc, in_maps, core_ids=list(range(B)))
    return np.stack([res.results[c]["out"] for c in range(B)])

